# revision 6
# baseline (speedup 1.0000x reference)
"""Trainium2 Bass kernel for nn_DynamicWeightAttention (v2).

Reference computation (per token t = (bt, n, h)):
    fused = concat(dyn[bt,n,h,:], static[n,h,:])            # C=32
    normed = LayerNorm(fused; gamma, beta, eps=1e-4)
    hmid   = tanh(normed @ w1 + b1)                         # HID=64
    score  = hmid @ w2 + b2                                 # scalar
    out[bt,n,:] = softmax over h of score                   # H=16

Strategy (8 NeuronCores, data-sharded over N: core c owns n in [32c, 32c+32)):
  - The LN mean term is folded into the weights: with every rhs K-row
    carrying an inv-scaled feature (dyn AND static), using
    W32 = gamma*w1 - colsum(gamma*w1)/32 makes the matmul compute the
    fully normalized pre-tanh activation directly.  b1 + beta@w1 is
    applied as a per-partition bias on the tanh ACT op.  So mm1 weights
    are CONSTANT across n (wa for token slots 0,1 / wb for slots 2,3)
    and there is no aux-row machinery.
  - Slab column layout: 128 K-rows = 4 tokens x (16 dyn | 16 static),
    all inv-scaled (GPSIMD), assembled token-major then xbar-DMA
    transposed to feature-major rhs tiles.
  - Stats on DVE: bf16 sum tree + square + square tree; invstd via
    bit-trick rsqrt + 1 Newton step.
  - mm1: 2 bf16 K=128 N=512 matmuls per n -> psum [128,1024]; tanh on
    ACT (bias = b1p per partition) -> ht bf16; mm2: 2 accumulating
    matmuls per n (M=32 band per n-octet) -> one [128,512] score psum
    per chunk; softmax: one exp (ACT), v-fold adds (DVE), block-diag
    ones matmul denominators (PE), reciprocal (DVE), mult (GPSIMD);
    one output transpose + gpsimd shuffle + one cast-DMA per chunk.
  - Software pipelined: stats(c+2) is emitted before mm(c); the output
    tail (out-transpose/shuffle/DMA) lags one chunk so the sync queue
    never blocks the next chunk's slab transposes.
"""
import os

import numpy as np
import ml_dtypes

import concourse.bacc as bacc
import concourse.mybir as mybir
from concourse.tile import TileContext
from concourse.bass_utils import run_bass_kernel_spmd

F32 = mybir.dt.float32
BF16 = mybir.dt.bfloat16
U32 = mybir.dt.uint32
AT = mybir.AluOpType
AFT = mybir.ActivationFunctionType

B_T, N, H, PD, PS, HID = 1024, 256, 16, 16, 16, 64
NCORES = 8
NC_N = N // NCORES          # 32 n's per core
EPS = 1e-4
MAGIC = float(0x5F3759DF)
NH = 16                     # n's per half-chunk (staging granularity)

_cached = {}


def _host_prep(dynamic_features, static_features, ln_gamma, ln_beta, w1, b1, w2, b2):
    g = np.asarray(ln_gamma, np.float32)
    be = np.asarray(ln_beta, np.float32)
    w1 = np.asarray(w1, np.float32)
    b1 = np.asarray(b1, np.float32)
    w2v = np.asarray(w2, np.float32).reshape(HID)
    st = np.asarray(static_features, np.float32)

    w1g = w1 * g[:, None]                       # [32, 64]
    cw = w1g.sum(0)                             # [64]
    W32 = w1g - cw[None, :] / 32.0              # [32, 64] mean folded
    b1p = b1 + be @ w1                          # [64]

    wa = np.zeros((128, 128), np.float32)
    wb = np.zeros((128, 128), np.float32)
    for a in range(2):
        wa[32 * a:32 * a + 32, 64 * a:64 * a + 64] = W32
        wb[32 * (a + 2):32 * (a + 2) + 32, 64 * a:64 * a + 64] = W32

    # mm2 weights: [w, n', 128, 32]; w=0 covers token slots 0,1 (ht cols
    # 0:512), w=1 slots 2,3.  Score row within band = 4*n' + a.
    m2w = np.zeros((2, 8, 128, 32), np.float32)
    for w in range(2):
        for npr in range(8):
            for ap in range(2):
                m2w[w, npr, 64 * ap:64 * ap + 64, 4 * npr + 2 * w + ap] = w2v

    ones4 = np.zeros((128, 128), np.float32)
    for k in range(128):
        ones4[k, (k // 4) * 4:(k // 4) * 4 + 4] = 1.0

    biasv = np.tile(b1p, 2).reshape(128, 1).astype(np.float32)

    s_st = st.sum(-1)                           # [256, 16]
    q_st = (st ** 2).sum(-1)                    # [256, 16]

    per_core = []
    for c in range(NCORES):
        stc = st[c * NC_N:(c + 1) * NC_N]       # [32, 16, 16]
        per_core.append({
            "dyn": np.ascontiguousarray(
                np.asarray(dynamic_features, np.float32)[:, c * NC_N:(c + 1) * NC_N]),
            "wa": wa.astype(ml_dtypes.bfloat16),
            "wb": wb.astype(ml_dtypes.bfloat16),
            "m2": m2w.astype(ml_dtypes.bfloat16),
            "ones4": ones4.astype(ml_dtypes.bfloat16),
            "bias": biasv,
            "st32": np.ascontiguousarray(stc.reshape(1, -1).astype(ml_dtypes.bfloat16)),
            "sst32": np.ascontiguousarray(
                (s_st[c * NC_N:(c + 1) * NC_N] / 32.0).reshape(1, 512).astype(np.float32)),
            "qst32": np.ascontiguousarray(
                (q_st[c * NC_N:(c + 1) * NC_N] / 32.0 + EPS).reshape(1, 512).astype(np.float32)),
        })
    return per_core


def build_nc(n_chunks=8):
    nc = bacc.Bacc("TRN2", target_bir_lowering=False, debug=False, num_devices=NCORES)
    dyn = nc.dram_tensor("dyn", [B_T, NC_N, H, PD], F32, kind="ExternalInput")
    wa_d = nc.dram_tensor("wa", [128, 128], BF16, kind="ExternalInput")
    wb_d = nc.dram_tensor("wb", [128, 128], BF16, kind="ExternalInput")
    m2_d = nc.dram_tensor("m2", [2, 8, 128, 32], BF16, kind="ExternalInput")
    ones_d = nc.dram_tensor("ones4", [128, 128], BF16, kind="ExternalInput")
    bias_d = nc.dram_tensor("bias", [128, 1], F32, kind="ExternalInput")
    st_d = nc.dram_tensor("st32", [1, NC_N * H * PS], BF16, kind="ExternalInput")
    sst_d = nc.dram_tensor("sst32", [1, 512], F32, kind="ExternalInput")
    qst_d = nc.dram_tensor("qst32", [1, 512], F32, kind="ExternalInput")
    out_d = nc.dram_tensor("out", [B_T, NC_N, H], F32, kind="ExternalOutput")

    dyn_v = dyn[:, :, :, :].rearrange("(p e) n h f -> p e n h f", e=8)
    out_v = out_d[:, :, :].rearrange("(p e) n h -> p e n h", e=8)

    with TileContext(nc) as tc:
        with tc.tile_pool(name="const", bufs=1) as cpool, \
             tc.tile_pool(name="stg", bufs=3) as stgpool, \
             tc.tile_pool(name="x2p", bufs=1) as x2pool, \
             tc.tile_pool(name="stats", bufs=1) as stpool, \
             tc.tile_pool(name="invp", bufs=2) as invpool, \
             tc.tile_pool(name="tr", bufs=2) as trpool, \
             tc.tile_pool(name="hid", bufs=3) as hpool, \
             tc.tile_pool(name="sm", bufs=2) as smpool, \
             tc.tile_pool(name="ot", bufs=2) as otpool, \
             tc.tile_pool(name="ps1", bufs=2, space="PSUM") as ps1pool, \
             tc.tile_pool(name="pss", bufs=2, space="PSUM") as psspool, \
             tc.tile_pool(name="psd", bufs=2, space="PSUM") as psdpool:

            # ---- constants / weights (loaded once) ----
            wat = cpool.tile([128, 128], BF16)
            nc.sync.dma_start(wat[:, :], wa_d[:, :])
            wbt = cpool.tile([128, 128], BF16)
            nc.sync.dma_start(wbt[:, :], wb_d[:, :])
            m2t = cpool.tile([128, 2, 8, 32], BF16)
            nc.sync.dma_start(m2t[:, :, :, :], m2_d[:, :, :, :].rearrange("w n k m -> k w n m"))
            onest = cpool.tile([128, 128], BF16)
            nc.sync.dma_start(onest[:, :], ones_d[:, :])
            biast = cpool.tile([128, 1], F32)
            nc.sync.dma_start(biast[:, :], bias_d[:, :])
            sstt = cpool.tile([128, 512], F32)
            nc.sync.dma_start(sstt[0:1, :], sst_d[:, :])
            nc.gpsimd.partition_broadcast(sstt[:, :], sstt[0:1, :], channels=128)
            qstt = cpool.tile([128, 512], F32)
            nc.sync.dma_start(qstt[0:1, :], qst_d[:, :])
            nc.gpsimd.partition_broadcast(qstt[:, :], qstt[0:1, :], channels=128)
            st32t = cpool.tile([128, NC_N * H * PS], BF16)
            nc.sync.dma_start(st32t[0:1, :], st_d[:, :])
            nc.gpsimd.partition_broadcast(st32t[:, :], st32t[0:1, :], channels=128)
            st32_v = st32t[:, :].rearrange("p (n h f) -> p n h f", n=NC_N, h=H)

            # ---- persistent slab buffers (3, manually rotated) ----
            # slab free layout (n, v, c) with c = 32*a + [16 dyn | 16 static]
            slabs = []
            for i in range(3):
                sl = cpool.tile([128, NC_N, 4, 128], BF16, tag=f"slab{i}")
                slabs.append(sl)

            def stats_phase(b8):
                """Load chunk b8, compute invstd, fill slab with scaled feats."""
                sl = slabs[b8 % 3]
                ssum = stpool.tile([128, 512], F32, tag="ssum")
                q = stpool.tile([128, 512], F32, tag="q")
                stgs = []
                for hc in range(2):
                    n0 = hc * NH
                    stg = stgpool.tile([128, NH, H, PD], BF16, tag="stg")
                    stgs.append(stg)
                    nc.gpsimd.dma_start(stg[:, :, :, :], dyn_v[:, b8, n0:n0 + NH, :, :])
                    stg_f = stg[:, :, :, :].rearrange("p n h f -> p (n h) f")

                    t8 = stpool.tile([128, 256, 8], BF16, tag="t8")
                    nc.vector.tensor_tensor(t8[:, :, :], stg_f[:, :, 0:8], stg_f[:, :, 8:16], AT.add)
                    t4 = stpool.tile([128, 256, 4], BF16, tag="t4")
                    nc.vector.tensor_tensor(t4[:, :, :], t8[:, :, 0:4], t8[:, :, 4:8], AT.add)
                    t2 = stpool.tile([128, 256, 2], BF16, tag="t2")
                    nc.vector.tensor_tensor(t2[:, :, :], t4[:, :, 0:2], t4[:, :, 2:4], AT.add)
                    nc.vector.tensor_tensor(ssum[:, n0 * 16:(n0 + NH) * 16],
                                            t2[:, :, 0], t2[:, :, 1], AT.add)

                    x2 = x2pool.tile([128, 256, 16], BF16, tag="x2")
                    nc.vector.tensor_tensor(x2[:, :, :], stg_f, stg_f, AT.mult)
                    q8 = stpool.tile([128, 256, 8], BF16, tag="t8")
                    nc.vector.tensor_tensor(q8[:, :, :], x2[:, :, 0:8], x2[:, :, 8:16], AT.add)
                    q4 = stpool.tile([128, 256, 4], BF16, tag="t4")
                    nc.vector.tensor_tensor(q4[:, :, :], q8[:, :, 0:4], q8[:, :, 4:8], AT.add)
                    q2 = stpool.tile([128, 256, 2], BF16, tag="t2")
                    nc.vector.tensor_tensor(q2[:, :, :], q4[:, :, 0:2], q4[:, :, 2:4], AT.add)
                    nc.vector.tensor_tensor(q[:, n0 * 16:(n0 + NH) * 16],
                                            q2[:, :, 0], q2[:, :, 1], AT.add)

                # stats chain on [128, 512] (n, h)
                mean = stpool.tile([128, 512], F32, tag="mean")
                nc.vector.scalar_tensor_tensor(mean[:, :], ssum[:, :], 1.0 / 32, sstt[:, :], AT.mult, AT.add)
                vareps = stpool.tile([128, 512], F32, tag="vareps")
                nc.vector.scalar_tensor_tensor(vareps[:, :], q[:, :], 1.0 / 32, qstt[:, :], AT.mult, AT.add)
                m2neg = stpool.tile([128, 512], F32, tag="m2neg")
                nc.vector.scalar_tensor_tensor(m2neg[:, :], mean[:, :], -1.0, mean[:, :], AT.mult, AT.mult)
                nc.vector.tensor_tensor(vareps[:, :], vareps[:, :], m2neg[:, :], AT.add)

                # invstd: bit-trick rsqrt seed + 1 Newton step
                seed = stpool.tile([128, 512], U32, tag="seed")
                nc.vector.tensor_scalar(seed[:, :], vareps[:, :].bitcast(U32), 1, None, AT.logical_shift_right)
                nc.vector.tensor_scalar(seed[:, :], seed[:, :], -1.0, MAGIC, AT.mult, AT.add)
                inv = invpool.tile([128, 512], F32, tag="inv")
                tmp = stpool.tile([128, 512], F32, tag="tmp")
                y0 = seed[:, :].bitcast(F32)
                nc.vector.tensor_tensor(tmp[:, :], y0, y0, AT.mult)
                nc.vector.scalar_tensor_tensor(tmp[:, :], tmp[:, :], -0.5, vareps[:, :], AT.mult, AT.mult)
                nc.vector.tensor_scalar(tmp[:, :], tmp[:, :], 1.5, None, AT.add)
                nc.vector.tensor_tensor(inv[:, :], y0, tmp[:, :], AT.mult)

                inv_nva = inv[:, :].rearrange("p (n v a) -> p n v a", n=NC_N, v=4)
                # scale features into the slab (GPSIMD)
                sl_v = sl[:, :, :, :].rearrange("p n v (a c) -> p n v a c", a=4)
                for hc in range(2):
                    n0 = hc * NH
                    inv_h = (inv_nva[:, n0:n0 + NH, :, :]
                             .rearrange("p n v (a o) -> p n v a o", o=1)
                             .broadcast_to([128, NH, 4, 4, 16]))
                    nc.gpsimd.tensor_tensor(
                        sl_v[:, n0:n0 + NH, :, :, 0:16],
                        stgs[hc][:, :, :, :].rearrange("p n (v a) f -> p n v a f", v=4),
                        inv_h, AT.mult)
                    nc.gpsimd.tensor_tensor(
                        sl_v[:, n0:n0 + NH, :, :, 16:32],
                        st32_v[:, n0:n0 + NH, :, :].rearrange("p n (v a) f -> p n v a f", v=4),
                        inv_h, AT.mult)

            pend_out = [None]

            def emit_out_tail(b8):
                """Output tail for chunk b8: transpose + shuffle + DMA."""
                ft = pend_out[0]
                otc = otpool.tile([128, 4, 128], BF16, tag="otc")
                nc.sync.dma_start_transpose(otc[:, :, :],
                                            ft[:, :, :].rearrange("p v c -> p (v c)"))
                # otc [p, v, m=(b, n', a)] -> otc2 [p, (b, n', v, a)] = [p, n, h]
                otc2 = otpool.tile([128, 512], BF16, tag="otc2")
                nc.gpsimd.tensor_copy(
                    otc2[:, :].rearrange("p (b np v a) -> p b np v a", b=4, np=8, v=4),
                    otc[:, :, :].rearrange("p v (b np a) -> p b np v a", b=4, np=8))
                nc.gpsimd.dma_start(out_v[:, b8, :, :],
                                    otc2[:, :].rearrange("p (n h) -> p n h", n=NC_N))

            def mm_phase(b8):
                """Transpose slab b8 and run mm1/tanh/mm2/softmax."""
                sl = slabs[b8 % 3]
                scores = psspool.tile([128, 512], F32, tag="scores")
                trts = []
                # all slab transposes up-front on the sync queue
                for nb in range(8):
                    trt = trpool.tile([128, 4, 4, 128], BF16, tag="tr")
                    trts.append(trt)
                    nc.sync.dma_start_transpose(
                        trt[:, :, :, :].rearrange("p n v c -> p (n v) c"),
                        sl[:, nb * 4:(nb + 1) * 4, :, :].rearrange("p n v c -> p (n v c)"))
                # previous chunk's output tail (sync queue after transposes)
                if pend_out[0] is not None:
                    emit_out_tail(b8 - 1)
                    pend_out[0] = None

                pend_mm2 = [None]

                def emit_mm2(n, ht):
                    b, npr = n // 8, n % 8
                    nc.tensor.matmul(scores[32 * b:32 * b + 32, :],
                                     m2t[:, 0, npr, :], ht[:, 0:512],
                                     start=(npr == 0), stop=False,
                                     tile_position=(0, 32 * b))
                    nc.tensor.matmul(scores[32 * b:32 * b + 32, :],
                                     m2t[:, 1, npr, :], ht[:, 512:1024],
                                     start=False, stop=(npr == 7),
                                     tile_position=(0, 32 * b))

                for nb in range(8):
                    trt = trts[nb]
                    for nl in range(4):
                        n = nb * 4 + nl
                        rhs = trt[:, nl, :, :].rearrange("p v c -> p (v c)")
                        ps = ps1pool.tile([128, 1024], F32, tag="ps1")
                        nc.tensor.matmul(ps[:, 0:512], wat[:, :], rhs, start=True, stop=True)
                        nc.tensor.matmul(ps[:, 512:1024], wbt[:, :], rhs, start=True, stop=True)
                        if pend_mm2[0] is not None:
                            emit_mm2(*pend_mm2[0])
                        ht = hpool.tile([128, 1024], BF16, tag="h")
                        nc.scalar.activation(ht[:, :], ps[:, :], AFT.Tanh,
                                             bias=biast[:, 0:1], scale=1.0)
                        pend_mm2[0] = (n, ht)
                emit_mm2(*pend_mm2[0])

                # softmax over h for the whole chunk
                et = smpool.tile([128, 4, 128], BF16, tag="e")
                nc.scalar.activation(et[:, :, :], scores[:, :].rearrange("p (v c) -> p v c", v=4), AFT.Exp)
                d1 = smpool.tile([128, 128], BF16, tag="d1")
                nc.vector.tensor_tensor(d1[:, :], et[:, 0, :], et[:, 1, :], AT.add)
                d2 = smpool.tile([128, 128], BF16, tag="d2")
                nc.vector.tensor_tensor(d2[:, :], et[:, 2, :], et[:, 3, :], AT.add)
                den = smpool.tile([128, 128], BF16, tag="den")
                nc.vector.tensor_tensor(den[:, :], d1[:, :], d2[:, :], AT.add)
                dps = psdpool.tile([128, 128], F32, tag="dps")
                nc.tensor.matmul(dps[:, :], onest[:, :], den[:, :], start=True, stop=True)
                rt = smpool.tile([128, 128], F32, tag="rt")
                nc.vector.reciprocal_approx_fast(rt[:, :], dps[:, :])
                ft = otpool.tile([128, 4, 128], BF16, tag="ft")
                nc.gpsimd.tensor_tensor(
                    ft[:, :, :], et[:, :, :],
                    rt[:, :].rearrange("p (o c) -> p o c", o=1).broadcast_to([128, 4, 128]),
                    AT.mult)
                pend_out[0] = ft

            # software pipeline, depth 2: stats(c+2) emitted before mm(c)
            stats_phase(0)
            if n_chunks > 1:
                stats_phase(1)
            for b8 in range(n_chunks):
                if b8 + 2 < n_chunks:
                    stats_phase(b8 + 2)
                mm_phase(b8)
            if pend_out[0] is not None:
                emit_out_tail(n_chunks - 1)
    nc.compile()
    return nc


def kernel(**inputs):
    per_core = _host_prep(**inputs)
    if "nc" not in _cached:
        _cached["nc"] = build_nc()
    nc = _cached["nc"]
    trace = bool(os.environ.get("DWA_TRACE"))
    res = run_bass_kernel_spmd(nc, per_core, core_ids=list(range(NCORES)), trace=trace)
    if trace:
        print("HW exec time:", res.exec_time_ns, "ns")
        kernel.last_result = res
    out = np.empty((B_T, N, H), np.float32)
    for c in range(NCORES):
        out[:, c * NC_N:(c + 1) * NC_N, :] = res.results[c]["out"]
    return out


# revision 10
# speedup vs baseline: 1.1335x; 1.1335x over previous
"""Trainium2 Bass kernel for nn_DynamicWeightAttention (v3).

Reference computation (per token t = (bt, n, h)):
    fused = concat(dyn[bt,n,h,:], static[n,h,:])            # C=32
    normed = LayerNorm(fused; gamma, beta, eps=1e-4)
    hmid   = tanh(normed @ w1 + b1)                         # HID=64
    score  = hmid @ w2 + b2                                 # scalar
    out[bt,n,:] = softmax over h of score                   # H=16

Strategy (8 NeuronCores, data-sharded over N: core c owns n in [32c, 32c+32)):
  - LN mean folded into weights (every rhs K-row carries an inv-scaled
    feature, so W32 = gamma*w1 - colsum(gamma*w1)/32 yields the
    normalized pre-tanh activation directly); b1 + beta@w1 applied as a
    per-partition ACT bias.  mm1 weights constant across n.
  - Slab column layout: 128 K-rows = [4 tokens x 16 dyn | 4 tokens x 16
    static], both halves inv-scaled by GPSIMD with contiguous 64-col
    writes, then xbar-DMA transposed to feature-major rhs tiles.
  - Stats on DVE: bf16 add trees ending in a single-port tensor_reduce;
    invstd via bit-trick rsqrt + 1 Newton step.
  - mm1: 2 bf16 K=128 N=512 matmuls per n -> psum [128,1024]; tanh on
    ACT (per-partition bias) -> ht bf16; mm2: 2 accumulating matmuls
    per n (M=32 band per n-octet) -> one [128,512] score psum per
    chunk; softmax: one exp (ACT), denominator entirely on PE (4
    accumulating block-diag ones matmuls over the v-slices),
    reciprocal (DVE), normalize (GPSIMD), one output transpose +
    gpsimd shuffle + one cast-DMA per chunk.
  - Emission phasing per iteration c: softmax-tail(c-1), output(c-2),
    stats(c+2), matmul-body(c) — so every engine queue head is ready
    and no tail op is trapped behind a later chunk's stats.
  - All DMAs are issued from the sync queue (HWDGE) so descriptor
    generation never contends with the DVE/GPSIMD shared SBUF port.
"""
import os

import numpy as np
import ml_dtypes

import concourse.bacc as bacc
import concourse.mybir as mybir
from concourse.tile import TileContext
from concourse.bass_utils import run_bass_kernel_spmd

F32 = mybir.dt.float32
BF16 = mybir.dt.bfloat16
U32 = mybir.dt.uint32
AT = mybir.AluOpType
AFT = mybir.ActivationFunctionType

B_T, N, H, PD, PS, HID = 1024, 256, 16, 16, 16, 64
NCORES = 8
NC_N = N // NCORES          # 32 n's per core
EPS = 1e-4
MAGIC = float(0x5F3759DF)
NH = 16                     # n's per half-chunk (staging granularity)

_cached = {}


def _host_prep(dynamic_features, static_features, ln_gamma, ln_beta, w1, b1, w2, b2):
    g = np.asarray(ln_gamma, np.float32)
    be = np.asarray(ln_beta, np.float32)
    w1 = np.asarray(w1, np.float32)
    b1 = np.asarray(b1, np.float32)
    w2v = np.asarray(w2, np.float32).reshape(HID)
    st = np.asarray(static_features, np.float32)

    w1g = w1 * g[:, None]                       # [32, 64]
    cw = w1g.sum(0)                             # [64]
    W32 = w1g - cw[None, :] / 32.0              # [32, 64] mean folded
    b1p = b1 + be @ w1                          # [64]
    Wdyn, Wst = W32[:PD], W32[PD:]

    # K-row layout: rows 16a+[0:16) = dyn of token a; rows 64+16a+[0:16)
    # = static of token a.  wa covers tokens 0,1 (M cols 0:64 / 64:128),
    # wb covers tokens 2,3.
    wa = np.zeros((128, 128), np.float32)
    wb = np.zeros((128, 128), np.float32)
    for a in range(2):
        wa[16 * a:16 * a + 16, 64 * a:64 * a + 64] = Wdyn
        wa[64 + 16 * a:64 + 16 * a + 16, 64 * a:64 * a + 64] = Wst
        wb[16 * (a + 2):16 * (a + 2) + 16, 64 * a:64 * a + 64] = Wdyn
        wb[64 + 16 * (a + 2):64 + 16 * (a + 2) + 16, 64 * a:64 * a + 64] = Wst

    # mm2 weights: [w, n', 128, 32]; w=0 covers token slots 0,1 (ht cols
    # 0:512), w=1 slots 2,3.  Score row within band = 4*n' + a.
    m2w = np.zeros((2, 8, 128, 32), np.float32)
    for w in range(2):
        for npr in range(8):
            for ap in range(2):
                m2w[w, npr, 64 * ap:64 * ap + 64, 4 * npr + 2 * w + ap] = w2v

    ones4 = np.zeros((128, 128), np.float32)
    for k in range(128):
        ones4[k, (k // 4) * 4:(k // 4) * 4 + 4] = 1.0

    biasv = np.tile(b1p, 2).reshape(128, 1).astype(np.float32)

    s_st = st.sum(-1)                           # [256, 16]
    q_st = (st ** 2).sum(-1)                    # [256, 16]

    per_core = []
    for c in range(NCORES):
        stc = st[c * NC_N:(c + 1) * NC_N]       # [32, 16, 16]
        per_core.append({
            "dyn": np.ascontiguousarray(
                np.asarray(dynamic_features, np.float32)[:, c * NC_N:(c + 1) * NC_N]),
            "wa": wa.astype(ml_dtypes.bfloat16),
            "wb": wb.astype(ml_dtypes.bfloat16),
            "m2": m2w.astype(ml_dtypes.bfloat16),
            "ones4": ones4.astype(ml_dtypes.bfloat16),
            "bias": biasv,
            "st32": np.ascontiguousarray(stc.reshape(1, -1).astype(ml_dtypes.bfloat16)),
            "sst32": np.ascontiguousarray(
                (s_st[c * NC_N:(c + 1) * NC_N] / 32.0).reshape(1, 512).astype(np.float32)),
            "qst32": np.ascontiguousarray(
                (q_st[c * NC_N:(c + 1) * NC_N] / 32.0 + EPS).reshape(1, 512).astype(np.float32)),
        })
    return per_core


def build_nc(n_chunks=8):
    nc = bacc.Bacc("TRN2", target_bir_lowering=False, debug=False, num_devices=NCORES)
    dyn = nc.dram_tensor("dyn", [B_T, NC_N, H, PD], F32, kind="ExternalInput")
    wa_d = nc.dram_tensor("wa", [128, 128], BF16, kind="ExternalInput")
    wb_d = nc.dram_tensor("wb", [128, 128], BF16, kind="ExternalInput")
    m2_d = nc.dram_tensor("m2", [2, 8, 128, 32], BF16, kind="ExternalInput")
    ones_d = nc.dram_tensor("ones4", [128, 128], BF16, kind="ExternalInput")
    bias_d = nc.dram_tensor("bias", [128, 1], F32, kind="ExternalInput")
    st_d = nc.dram_tensor("st32", [1, NC_N * H * PS], BF16, kind="ExternalInput")
    sst_d = nc.dram_tensor("sst32", [1, 512], F32, kind="ExternalInput")
    qst_d = nc.dram_tensor("qst32", [1, 512], F32, kind="ExternalInput")
    out_d = nc.dram_tensor("out", [B_T, NC_N, H], F32, kind="ExternalOutput")

    dyn_v = dyn[:, :, :, :].rearrange("(p e) n h f -> p e n h f", e=8)
    out_v = out_d[:, :, :].rearrange("(p e) n h -> p e n h", e=8)

    with TileContext(nc) as tc:
        with tc.tile_pool(name="const", bufs=1) as cpool, \
             tc.tile_pool(name="stg", bufs=3) as stgpool, \
             tc.tile_pool(name="x2p", bufs=1) as x2pool, \
             tc.tile_pool(name="stats", bufs=1) as stpool, \
             tc.tile_pool(name="invp", bufs=2) as invpool, \
             tc.tile_pool(name="tr", bufs=2) as trpool, \
             tc.tile_pool(name="hid", bufs=3) as hpool, \
             tc.tile_pool(name="sm", bufs=2) as smpool, \
             tc.tile_pool(name="ot", bufs=2) as otpool, \
             tc.tile_pool(name="ps1", bufs=2, space="PSUM") as ps1pool, \
             tc.tile_pool(name="pss", bufs=2, space="PSUM") as psspool, \
             tc.tile_pool(name="psd", bufs=2, space="PSUM") as psdpool:

            # ---- constants / weights (loaded once) ----
            wat = cpool.tile([128, 128], BF16)
            nc.sync.dma_start(wat[:, :], wa_d[:, :])
            wbt = cpool.tile([128, 128], BF16)
            nc.sync.dma_start(wbt[:, :], wb_d[:, :])
            m2t = cpool.tile([128, 2, 8, 32], BF16)
            nc.sync.dma_start(m2t[:, :, :, :], m2_d[:, :, :, :].rearrange("w n k m -> k w n m"))
            onest = cpool.tile([128, 128], BF16)
            nc.sync.dma_start(onest[:, :], ones_d[:, :])
            biast = cpool.tile([128, 1], F32)
            nc.sync.dma_start(biast[:, :], bias_d[:, :])
            sstt = cpool.tile([128, 512], F32)
            nc.sync.dma_start(sstt[0:1, :], sst_d[:, :])
            nc.gpsimd.partition_broadcast(sstt[:, :], sstt[0:1, :], channels=128)
            qstt = cpool.tile([128, 512], F32)
            nc.sync.dma_start(qstt[0:1, :], qst_d[:, :])
            nc.gpsimd.partition_broadcast(qstt[:, :], qstt[0:1, :], channels=128)
            st32t = cpool.tile([128, NC_N * H * PS], BF16)
            nc.sync.dma_start(st32t[0:1, :], st_d[:, :])
            nc.gpsimd.partition_broadcast(st32t[:, :], st32t[0:1, :], channels=128)
            st32_v = st32t[:, :].rearrange("p (n h f) -> p n h f", n=NC_N, h=H)

            # ---- persistent slab buffers (3, manually rotated) ----
            # slab free layout (n, v, c): c = [64: dyn (a,f) | 64: static (a,f)]
            slabs = []
            for i in range(3):
                sl = cpool.tile([128, NC_N, 4, 128], BF16, tag=f"slab{i}")
                slabs.append(sl)

            def stats_phase(b8):
                """Load chunk b8, compute invstd, fill slab with scaled feats."""
                sl = slabs[b8 % 3]
                ssum = stpool.tile([128, 512], F32, tag="ssum")
                q = stpool.tile([128, 512], F32, tag="q")
                stgs = []
                for hc in range(2):
                    n0 = hc * NH
                    stg = stgpool.tile([128, NH, H, PD], BF16, tag="stg")
                    stgs.append(stg)
                    nc.gpsimd.dma_start(stg[:, :, :, :], dyn_v[:, b8, n0:n0 + NH, :, :])
                    stg_f = stg[:, :, :, :].rearrange("p n h f -> p (n h) f")

                    t8 = stpool.tile([128, 256, 8], BF16, tag="t8")
                    nc.vector.tensor_tensor(t8[:, :, :], stg_f[:, :, 0:8], stg_f[:, :, 8:16], AT.add)
                    t4 = stpool.tile([128, 256, 4], BF16, tag="t4")
                    nc.vector.tensor_tensor(t4[:, :, :], t8[:, :, 0:4], t8[:, :, 4:8], AT.add)
                    nc.vector.tensor_reduce(ssum[:, n0 * 16:(n0 + NH) * 16], t4[:, :, :],
                                            axis=mybir.AxisListType.X, op=AT.add)

                    x2 = x2pool.tile([128, 256, 16], BF16, tag="x2")
                    nc.vector.tensor_tensor(x2[:, :, :], stg_f, stg_f, AT.mult)
                    q8 = stpool.tile([128, 256, 8], BF16, tag="t8")
                    nc.vector.tensor_tensor(q8[:, :, :], x2[:, :, 0:8], x2[:, :, 8:16], AT.add)
                    q4 = stpool.tile([128, 256, 4], BF16, tag="t4")
                    nc.vector.tensor_tensor(q4[:, :, :], q8[:, :, 0:4], q8[:, :, 4:8], AT.add)
                    nc.vector.tensor_reduce(q[:, n0 * 16:(n0 + NH) * 16], q4[:, :, :],
                                            axis=mybir.AxisListType.X, op=AT.add)

                # stats chain on [128, 512] (n, h)
                mean = stpool.tile([128, 512], F32, tag="mean")
                nc.vector.scalar_tensor_tensor(mean[:, :], ssum[:, :], 1.0 / 32, sstt[:, :], AT.mult, AT.add)
                vareps = stpool.tile([128, 512], F32, tag="vareps")
                nc.vector.scalar_tensor_tensor(vareps[:, :], q[:, :], 1.0 / 32, qstt[:, :], AT.mult, AT.add)
                m2neg = stpool.tile([128, 512], F32, tag="m2neg")
                nc.vector.scalar_tensor_tensor(m2neg[:, :], mean[:, :], -1.0, mean[:, :], AT.mult, AT.mult)
                nc.vector.tensor_tensor(vareps[:, :], vareps[:, :], m2neg[:, :], AT.add)

                # invstd: bit-trick rsqrt seed + 1 Newton step
                seed = stpool.tile([128, 512], U32, tag="seed")
                nc.vector.tensor_scalar(seed[:, :], vareps[:, :].bitcast(U32), 1, None, AT.logical_shift_right)
                nc.vector.tensor_scalar(seed[:, :], seed[:, :], -1.0, MAGIC, AT.mult, AT.add)
                inv = invpool.tile([128, 512], F32, tag="inv")
                tmp = stpool.tile([128, 512], F32, tag="tmp")
                y0 = seed[:, :].bitcast(F32)
                nc.vector.tensor_tensor(tmp[:, :], y0, y0, AT.mult)
                nc.vector.scalar_tensor_tensor(tmp[:, :], tmp[:, :], -0.5, vareps[:, :], AT.mult, AT.mult)
                nc.vector.tensor_scalar(tmp[:, :], tmp[:, :], 1.5, None, AT.add)
                nc.vector.tensor_tensor(inv[:, :], y0, tmp[:, :], AT.mult)

                inv_nva = inv[:, :].rearrange("p (n v a) -> p n v a", n=NC_N, v=4)
                # scale features into the slab (GPSIMD, contiguous 64-col writes)
                sl_d = sl[:, :, :, :].rearrange("p n v (g a c) -> p n v g a c", g=2, a=4)
                for hc in range(2):
                    n0 = hc * NH
                    inv_h = (inv_nva[:, n0:n0 + NH, :, :]
                             .rearrange("p n v (a o) -> p n v a o", o=1)
                             .broadcast_to([128, NH, 4, 4, 16]))
                    nc.gpsimd.tensor_tensor(
                        sl_d[:, n0:n0 + NH, :, 0, :, :],
                        stgs[hc][:, :, :, :].rearrange("p n (v a) f -> p n v a f", v=4),
                        inv_h, AT.mult)
                    nc.gpsimd.tensor_tensor(
                        sl_d[:, n0:n0 + NH, :, 1, :, :],
                        st32_v[:, n0:n0 + NH, :, :].rearrange("p n (v a) f -> p n v a f", v=4),
                        inv_h, AT.mult)

            pend_sm = [None]    # scores psum of chunk awaiting softmax tail
            pend_out = [None]   # ft tile of chunk awaiting output

            def softmax_tail(b8):
                """exp -> denominators (PE) -> recip -> normalize for chunk b8."""
                scores = pend_sm[0]
                pend_sm[0] = None
                et = smpool.tile([128, 4, 128], BF16, tag="e")
                nc.scalar.activation(et[:, :, :],
                                     scores[:, :].rearrange("p (v c) -> p v c", v=4), AFT.Exp)
                dps = psdpool.tile([128, 128], F32, tag="dps")
                for v in range(4):
                    nc.tensor.matmul(dps[:, :], onest[:, :], et[:, v, :],
                                     start=(v == 0), stop=(v == 3))
                rt = smpool.tile([128, 128], F32, tag="rt")
                nc.vector.reciprocal_approx_fast(rt[:, :], dps[:, :])
                ft = otpool.tile([128, 4, 128], BF16, tag="ft")
                nc.gpsimd.tensor_tensor(
                    ft[:, :, :], et[:, :, :],
                    rt[:, :].rearrange("p (o c) -> p o c", o=1).broadcast_to([128, 4, 128]),
                    AT.mult)
                pend_out[0] = (b8, ft)

            def emit_out(b8, ft):
                """Output for chunk b8: transpose + shuffle + DMA."""
                otc = otpool.tile([128, 4, 128], BF16, tag="otc")
                nc.sync.dma_start_transpose(otc[:, :, :],
                                            ft[:, :, :].rearrange("p v c -> p (v c)"))
                # otc [p, v, m=(b, n', a)] -> otc2 [p, (b, n', v, a)] = [p, n, h]
                otc2 = otpool.tile([128, 512], BF16, tag="otc2")
                nc.gpsimd.tensor_copy(
                    otc2[:, :].rearrange("p (b np v a) -> p b np v a", b=4, np=8, v=4),
                    otc[:, :, :].rearrange("p v (b np a) -> p b np v a", b=4, np=8))
                nc.gpsimd.dma_start(out_v[:, b8, :, :],
                                    otc2[:, :].rearrange("p (n h) -> p n h", n=NC_N))

            def mm_body(b8):
                """Transpose slab b8 and run mm1/tanh/mm2."""
                sl = slabs[b8 % 3]
                scores = psspool.tile([128, 512], F32, tag="scores")
                trts = []
                for nb in range(8):
                    trt = trpool.tile([128, 4, 4, 128], BF16, tag="tr")
                    trts.append(trt)
                    nc.sync.dma_start_transpose(
                        trt[:, :, :, :].rearrange("p n v c -> p (n v) c"),
                        sl[:, nb * 4:(nb + 1) * 4, :, :].rearrange("p n v c -> p (n v c)"))

                pend_mm2 = [None]

                def emit_mm2(n, ht):
                    b, npr = n // 8, n % 8
                    nc.tensor.matmul(scores[32 * b:32 * b + 32, :],
                                     m2t[:, 0, npr, :], ht[:, 0:512],
                                     start=(npr == 0), stop=False,
                                     tile_position=(0, 32 * b))
                    nc.tensor.matmul(scores[32 * b:32 * b + 32, :],
                                     m2t[:, 1, npr, :], ht[:, 512:1024],
                                     start=False, stop=(npr == 7),
                                     tile_position=(0, 32 * b))

                for nb in range(8):
                    trt = trts[nb]
                    for nl in range(4):
                        n = nb * 4 + nl
                        rhs = trt[:, nl, :, :].rearrange("p v c -> p (v c)")
                        ps = ps1pool.tile([128, 1024], F32, tag="ps1")
                        nc.tensor.matmul(ps[:, 0:512], wat[:, :], rhs, start=True, stop=True)
                        nc.tensor.matmul(ps[:, 512:1024], wbt[:, :], rhs, start=True, stop=True)
                        if pend_mm2[0] is not None:
                            emit_mm2(*pend_mm2[0])
                        ht = hpool.tile([128, 1024], BF16, tag="h")
                        nc.scalar.activation(ht[:, :], ps[:, :], AFT.Tanh,
                                             bias=biast[:, 0:1], scale=1.0)
                        pend_mm2[0] = (n, ht)
                emit_mm2(*pend_mm2[0])
                pend_sm[0] = scores

            # pipeline: per iteration c emit tail(c-1), out(c-2), stats(c+2), body(c)
            stats_phase(0)
            if n_chunks > 1:
                stats_phase(1)
            for b8 in range(n_chunks):
                if pend_out[0] is not None:
                    emit_out(*pend_out[0])
                    pend_out[0] = None
                if pend_sm[0] is not None:
                    softmax_tail(b8 - 1)
                if b8 + 2 < n_chunks:
                    stats_phase(b8 + 2)
                mm_body(b8)
            # drain: output(n-2), tail(n-1), output(n-1)
            if pend_out[0] is not None:
                emit_out(*pend_out[0])
                pend_out[0] = None
            softmax_tail(n_chunks - 1)
            emit_out(*pend_out[0])
            pend_out[0] = None
    nc.compile()
    return nc


def kernel(**inputs):
    per_core = _host_prep(**inputs)
    if "nc" not in _cached:
        _cached["nc"] = build_nc()
    nc = _cached["nc"]
    trace = bool(os.environ.get("DWA_TRACE"))
    res = run_bass_kernel_spmd(nc, per_core, core_ids=list(range(NCORES)), trace=trace)
    if trace:
        print("HW exec time:", res.exec_time_ns, "ns")
        kernel.last_result = res
    out = np.empty((B_T, N, H), np.float32)
    for c in range(NCORES):
        out[:, c * NC_N:(c + 1) * NC_N, :] = res.results[c]["out"]
    return out


# revision 11
# speedup vs baseline: 1.2507x; 1.1034x over previous
"""Trainium2 Bass kernel for nn_DynamicWeightAttention (v3).

Reference computation (per token t = (bt, n, h)):
    fused = concat(dyn[bt,n,h,:], static[n,h,:])            # C=32
    normed = LayerNorm(fused; gamma, beta, eps=1e-4)
    hmid   = tanh(normed @ w1 + b1)                         # HID=64
    score  = hmid @ w2 + b2                                 # scalar
    out[bt,n,:] = softmax over h of score                   # H=16

Strategy (8 NeuronCores, data-sharded over N: core c owns n in [32c, 32c+32)):
  - LN mean folded into weights (every rhs K-row carries an inv-scaled
    feature, so W32 = gamma*w1 - colsum(gamma*w1)/32 yields the
    normalized pre-tanh activation directly); b1 + beta@w1 applied as a
    per-partition ACT bias.  mm1 weights constant across n.
  - Slab column layout: 128 K-rows = [4 tokens x 16 dyn | 4 tokens x 16
    static], both halves inv-scaled by GPSIMD with contiguous 64-col
    writes, then xbar-DMA transposed to feature-major rhs tiles.
  - Stats on DVE: bf16 add trees ending in a single-port tensor_reduce;
    invstd via bit-trick rsqrt + 1 Newton step.
  - mm1: 2 bf16 K=128 N=512 matmuls per n -> psum [128,1024]; tanh on
    ACT (per-partition bias) -> ht bf16; mm2: 2 accumulating matmuls
    per n (M=32 band per n-octet) -> one [128,512] score psum per
    chunk; softmax: one exp (ACT), denominator entirely on PE (4
    accumulating block-diag ones matmuls over the v-slices),
    reciprocal (DVE), normalize (GPSIMD), one output transpose +
    gpsimd shuffle + one cast-DMA per chunk.
  - Emission phasing per iteration c: softmax-tail(c-1), output(c-2),
    stats(c+2), matmul-body(c) — so every engine queue head is ready
    and no tail op is trapped behind a later chunk's stats.
  - All DMAs are issued from the sync queue (HWDGE) so descriptor
    generation never contends with the DVE/GPSIMD shared SBUF port.
"""
import os

import numpy as np
import ml_dtypes

import concourse.bacc as bacc
import concourse.mybir as mybir
from concourse.tile import TileContext
from concourse.bass_utils import run_bass_kernel_spmd

F32 = mybir.dt.float32
BF16 = mybir.dt.bfloat16
U32 = mybir.dt.uint32
AT = mybir.AluOpType
AFT = mybir.ActivationFunctionType

B_T, N, H, PD, PS, HID = 1024, 256, 16, 16, 16, 64
NCORES = 8
NC_N = N // NCORES          # 32 n's per core
EPS = 1e-4
MAGIC = float(0x5F3759DF)
NH = 16                     # n's per half-chunk (staging granularity)

_cached = {}


def _host_prep(dynamic_features, static_features, ln_gamma, ln_beta, w1, b1, w2, b2):
    g = np.asarray(ln_gamma, np.float32)
    be = np.asarray(ln_beta, np.float32)
    w1 = np.asarray(w1, np.float32)
    b1 = np.asarray(b1, np.float32)
    w2v = np.asarray(w2, np.float32).reshape(HID)
    st = np.asarray(static_features, np.float32)

    w1g = w1 * g[:, None]                       # [32, 64]
    cw = w1g.sum(0)                             # [64]
    b1p = b1 + be @ w1                          # [64]
    Wdyn = w1g[:PD] - cw[None, :] / 32.0        # [16, 64] dyn + mean fold
    w1s = w1g[PD:]                              # [16, 64]

    s_st = st.sum(-1)                           # [256, 16]
    q_st = (st ** 2).sum(-1)                    # [256, 16]
    # static projection with its share of the mean fold
    sp = np.einsum("nhp,pd->nhd", st, w1s) - s_st[..., None] * cw[None, None, :] / 32.0

    # mm2 weights: [w, n', 128, 32]; w=0 covers token slots 0,1 (ht cols
    # 0:512), w=1 slots 2,3.  Score row within band = 4*n' + a.
    m2w = np.zeros((2, 8, 128, 32), np.float32)
    for w in range(2):
        for npr in range(8):
            for ap in range(2):
                m2w[w, npr, 64 * ap:64 * ap + 64, 4 * npr + 2 * w + ap] = w2v

    ones4 = np.zeros((128, 128), np.float32)
    for k in range(128):
        ones4[k, (k // 4) * 4:(k // 4) * 4 + 4] = 1.0

    biasv = np.tile(b1p, 2).reshape(128, 1).astype(np.float32)

    per_core = []
    for c in range(NCORES):
        spc = sp[c * NC_N:(c + 1) * NC_N]       # [32, 16, 64]
        # K-row layout per column: [0:64) dyn (16a+f), [64:80) sel
        # (64+4a+v'), [80:128) zero.  Column (n, v) token a has h=4v+a;
        # sel row fires (value inv) iff v'==v, weight = sp[n, 4v'+a].
        wa = np.zeros((NC_N, 128, 128), np.float32)
        wb = np.zeros((NC_N, 128, 128), np.float32)
        for n in range(NC_N):
            for a in range(4):
                tgt = wa if a < 2 else wb
                mcol = 64 * (a % 2)
                tgt[n, 16 * a:16 * a + 16, mcol:mcol + 64] = Wdyn
                for vp in range(4):
                    tgt[n, 64 + 4 * a + vp, mcol:mcol + 64] = spc[n, 4 * vp + a]
        per_core.append({
            "dyn": np.ascontiguousarray(
                np.asarray(dynamic_features, np.float32)[:, c * NC_N:(c + 1) * NC_N]),
            "wa": wa.astype(ml_dtypes.bfloat16),
            "wb": wb.astype(ml_dtypes.bfloat16),
            "m2": m2w.astype(ml_dtypes.bfloat16),
            "ones4": ones4.astype(ml_dtypes.bfloat16),
            "bias": biasv,
            "sst32": np.ascontiguousarray(
                (s_st[c * NC_N:(c + 1) * NC_N] / 32.0).reshape(1, 512).astype(np.float32)),
            "qst32": np.ascontiguousarray(
                (q_st[c * NC_N:(c + 1) * NC_N] / 32.0 + EPS).reshape(1, 512).astype(np.float32)),
        })
    return per_core


def build_nc(n_chunks=8):
    nc = bacc.Bacc("TRN2", target_bir_lowering=False, debug=False, num_devices=NCORES)
    dyn = nc.dram_tensor("dyn", [B_T, NC_N, H, PD], F32, kind="ExternalInput")
    wa_d = nc.dram_tensor("wa", [NC_N, 128, 128], BF16, kind="ExternalInput")
    wb_d = nc.dram_tensor("wb", [NC_N, 128, 128], BF16, kind="ExternalInput")
    m2_d = nc.dram_tensor("m2", [2, 8, 128, 32], BF16, kind="ExternalInput")
    ones_d = nc.dram_tensor("ones4", [128, 128], BF16, kind="ExternalInput")
    bias_d = nc.dram_tensor("bias", [128, 1], F32, kind="ExternalInput")
    sst_d = nc.dram_tensor("sst32", [1, 512], F32, kind="ExternalInput")
    qst_d = nc.dram_tensor("qst32", [1, 512], F32, kind="ExternalInput")
    out_d = nc.dram_tensor("out", [B_T, NC_N, H], F32, kind="ExternalOutput")

    dyn_v = dyn[:, :, :, :].rearrange("(p e) n h f -> p e n h f", e=8)
    out_v = out_d[:, :, :].rearrange("(p e) n h -> p e n h", e=8)

    with TileContext(nc) as tc:
        with tc.tile_pool(name="const", bufs=1) as cpool, \
             tc.tile_pool(name="stg", bufs=3) as stgpool, \
             tc.tile_pool(name="x2p", bufs=1) as x2pool, \
             tc.tile_pool(name="stats", bufs=1) as stpool, \
             tc.tile_pool(name="invp", bufs=2) as invpool, \
             tc.tile_pool(name="tr", bufs=2) as trpool, \
             tc.tile_pool(name="hid", bufs=3) as hpool, \
             tc.tile_pool(name="sm", bufs=2) as smpool, \
             tc.tile_pool(name="ot", bufs=2) as otpool, \
             tc.tile_pool(name="ps1", bufs=2, space="PSUM") as ps1pool, \
             tc.tile_pool(name="pss", bufs=2, space="PSUM") as psspool, \
             tc.tile_pool(name="psd", bufs=2, space="PSUM") as psdpool:

            # ---- constants / weights (loaded once) ----
            wat = cpool.tile([128, NC_N, 128], BF16)
            nc.sync.dma_start(wat[:, :, :], wa_d[:, :, :].rearrange("n k m -> k n m"))
            wbt = cpool.tile([128, NC_N, 128], BF16)
            nc.sync.dma_start(wbt[:, :, :], wb_d[:, :, :].rearrange("n k m -> k n m"))
            m2t = cpool.tile([128, 2, 8, 32], BF16)
            nc.sync.dma_start(m2t[:, :, :, :], m2_d[:, :, :, :].rearrange("w n k m -> k w n m"))
            onest = cpool.tile([128, 128], BF16)
            nc.sync.dma_start(onest[:, :], ones_d[:, :])
            biast = cpool.tile([128, 1], F32)
            nc.sync.dma_start(biast[:, :], bias_d[:, :])
            sstt = cpool.tile([128, 512], F32)
            nc.sync.dma_start(sstt[0:1, :], sst_d[:, :])
            nc.gpsimd.partition_broadcast(sstt[:, :], sstt[0:1, :], channels=128)
            qstt = cpool.tile([128, 512], F32)
            nc.sync.dma_start(qstt[0:1, :], qst_d[:, :])
            nc.gpsimd.partition_broadcast(qstt[:, :], qstt[0:1, :], channels=128)

            # ---- persistent slab buffers (3, manually rotated) ----
            # slab free layout (n, v, c): c = [64: dyn (a,f) | 64: static (a,f)]
            slabs = []
            for i in range(3):
                sl = cpool.tile([128, NC_N, 4, 128], BF16, tag=f"slab{i}")
                nc.vector.memset(sl[:, :, :, 64:128], 0.0)
                slabs.append(sl)

            def stats_phase(b8):
                """Load chunk b8, compute invstd, fill slab with scaled feats."""
                sl = slabs[b8 % 3]
                ssum = stpool.tile([128, 512], F32, tag="ssum")
                q = stpool.tile([128, 512], F32, tag="q")
                stgs = []
                for hc in range(2):
                    n0 = hc * NH
                    stg = stgpool.tile([128, NH, H, PD], BF16, tag="stg")
                    stgs.append(stg)
                    nc.gpsimd.dma_start(stg[:, :, :, :], dyn_v[:, b8, n0:n0 + NH, :, :])
                    stg_f = stg[:, :, :, :].rearrange("p n h f -> p (n h) f")

                    t8 = stpool.tile([128, 256, 8], BF16, tag="t8")
                    nc.vector.tensor_tensor(t8[:, :, :], stg_f[:, :, 0:8], stg_f[:, :, 8:16], AT.add)
                    t4 = stpool.tile([128, 256, 4], BF16, tag="t4")
                    nc.vector.tensor_tensor(t4[:, :, :], t8[:, :, 0:4], t8[:, :, 4:8], AT.add)
                    nc.vector.tensor_reduce(ssum[:, n0 * 16:(n0 + NH) * 16], t4[:, :, :],
                                            axis=mybir.AxisListType.X, op=AT.add)

                    x2 = x2pool.tile([128, 256, 16], BF16, tag="x2")
                    nc.vector.tensor_tensor(x2[:, :, :], stg_f, stg_f, AT.mult)
                    q8 = stpool.tile([128, 256, 8], BF16, tag="t8")
                    nc.vector.tensor_tensor(q8[:, :, :], x2[:, :, 0:8], x2[:, :, 8:16], AT.add)
                    q4 = stpool.tile([128, 256, 4], BF16, tag="t4")
                    nc.vector.tensor_tensor(q4[:, :, :], q8[:, :, 0:4], q8[:, :, 4:8], AT.add)
                    nc.vector.tensor_reduce(q[:, n0 * 16:(n0 + NH) * 16], q4[:, :, :],
                                            axis=mybir.AxisListType.X, op=AT.add)

                # stats chain on [128, 512] (n, h)
                mean = stpool.tile([128, 512], F32, tag="mean")
                nc.vector.scalar_tensor_tensor(mean[:, :], ssum[:, :], 1.0 / 32, sstt[:, :], AT.mult, AT.add)
                vareps = stpool.tile([128, 512], F32, tag="vareps")
                nc.vector.scalar_tensor_tensor(vareps[:, :], q[:, :], 1.0 / 32, qstt[:, :], AT.mult, AT.add)
                m2neg = stpool.tile([128, 512], F32, tag="m2neg")
                nc.vector.scalar_tensor_tensor(m2neg[:, :], mean[:, :], -1.0, mean[:, :], AT.mult, AT.mult)
                nc.vector.tensor_tensor(vareps[:, :], vareps[:, :], m2neg[:, :], AT.add)

                # invstd: bit-trick rsqrt seed + 1 Newton step
                seed = stpool.tile([128, 512], U32, tag="seed")
                nc.vector.tensor_scalar(seed[:, :], vareps[:, :].bitcast(U32), 1, None, AT.logical_shift_right)
                nc.vector.tensor_scalar(seed[:, :], seed[:, :], -1.0, MAGIC, AT.mult, AT.add)
                inv = invpool.tile([128, 512], F32, tag="inv")
                tmp = stpool.tile([128, 512], F32, tag="tmp")
                y0 = seed[:, :].bitcast(F32)
                nc.vector.tensor_tensor(tmp[:, :], y0, y0, AT.mult)
                nc.vector.scalar_tensor_tensor(tmp[:, :], tmp[:, :], -0.5, vareps[:, :], AT.mult, AT.mult)
                nc.vector.tensor_scalar(tmp[:, :], tmp[:, :], 1.5, None, AT.add)
                nc.vector.tensor_tensor(inv[:, :], y0, tmp[:, :], AT.mult)

                inv_nva = inv[:, :].rearrange("p (n v a) -> p n v a", n=NC_N, v=4)
                # sel diagonal: slab[p, n, v, 64 + 4a + v] = inv[p, n, 4v+a].
                # Off-diagonal sel slots are never written (position depends
                # on v), so the one-time init keeps them zero.
                slf = sl[:, :, :, :].rearrange("p n v c -> p (n v c)")
                p0 = list(slf.ap)[0]
                from concourse.ap import AP as BassAP
                nc.gpsimd.tensor_copy(
                    BassAP(slf.tensor, slf.offset + 64,
                           [p0, [512, NC_N], [129, 4], [4, 4]]),
                    inv_nva)
                # scale dyn features into the slab (contiguous 64-col writes)
                sl_d = sl[:, :, :, :].rearrange("p n v (a c) -> p n v a c", a=8)
                for hc in range(2):
                    n0 = hc * NH
                    inv_h = (inv_nva[:, n0:n0 + NH, :, :]
                             .rearrange("p n v (a o) -> p n v a o", o=1)
                             .broadcast_to([128, NH, 4, 4, 16]))
                    nc.gpsimd.tensor_tensor(
                        sl_d[:, n0:n0 + NH, :, 0:4, :],
                        stgs[hc][:, :, :, :].rearrange("p n (v a) f -> p n v a f", v=4),
                        inv_h, AT.mult)

            pend_sm = [None]    # scores psum of chunk awaiting softmax tail
            pend_out = [None]   # ft tile of chunk awaiting output

            def softmax_tail(b8):
                """exp -> denominators (PE) -> recip -> normalize for chunk b8."""
                scores = pend_sm[0]
                pend_sm[0] = None
                et = smpool.tile([128, 4, 128], BF16, tag="e")
                nc.scalar.activation(et[:, :, :],
                                     scores[:, :].rearrange("p (v c) -> p v c", v=4), AFT.Exp)
                dps = psdpool.tile([128, 128], F32, tag="dps")
                for v in range(4):
                    nc.tensor.matmul(dps[:, :], onest[:, :], et[:, v, :],
                                     start=(v == 0), stop=(v == 3))
                rt = smpool.tile([128, 128], F32, tag="rt")
                nc.vector.reciprocal_approx_fast(rt[:, :], dps[:, :])
                ft = otpool.tile([128, 4, 128], BF16, tag="ft")
                nc.gpsimd.tensor_tensor(
                    ft[:, :, :], et[:, :, :],
                    rt[:, :].rearrange("p (o c) -> p o c", o=1).broadcast_to([128, 4, 128]),
                    AT.mult)
                pend_out[0] = (b8, ft)

            def emit_out(b8, ft):
                """Output for chunk b8: transpose + shuffle + DMA."""
                otc = otpool.tile([128, 4, 128], BF16, tag="otc")
                nc.sync.dma_start_transpose(otc[:, :, :],
                                            ft[:, :, :].rearrange("p v c -> p (v c)"))
                # otc [p, v, m=(b, n', a)] -> otc2 [p, (b, n', v, a)] = [p, n, h]
                otc2 = otpool.tile([128, 512], BF16, tag="otc2")
                nc.scalar.copy(
                    otc2[:, :].rearrange("p (b np v a) -> p b np v a", b=4, np=8, v=4),
                    otc[:, :, :].rearrange("p v (b np a) -> p b np v a", b=4, np=8))
                nc.gpsimd.dma_start(out_v[:, b8, :, :],
                                    otc2[:, :].rearrange("p (n h) -> p n h", n=NC_N))

            def mm_body(b8):
                """Transpose slab b8 and run mm1/tanh/mm2."""
                sl = slabs[b8 % 3]
                scores = psspool.tile([128, 512], F32, tag="scores")
                trts = []
                for nb in range(8):
                    trt = trpool.tile([128, 4, 4, 128], BF16, tag="tr")
                    trts.append(trt)
                    nc.sync.dma_start_transpose(
                        trt[:, :, :, :].rearrange("p n v c -> p (n v) c"),
                        sl[:, nb * 4:(nb + 1) * 4, :, :].rearrange("p n v c -> p (n v c)"))

                pend_mm2 = [None]

                def emit_mm2(n, ht):
                    b, npr = n // 8, n % 8
                    nc.tensor.matmul(scores[32 * b:32 * b + 32, :],
                                     m2t[:, 0, npr, :], ht[:, 0:512],
                                     start=(npr == 0), stop=False,
                                     tile_position=(0, 32 * b))
                    nc.tensor.matmul(scores[32 * b:32 * b + 32, :],
                                     m2t[:, 1, npr, :], ht[:, 512:1024],
                                     start=False, stop=(npr == 7),
                                     tile_position=(0, 32 * b))

                for nb in range(8):
                    trt = trts[nb]
                    for nl in range(4):
                        n = nb * 4 + nl
                        rhs = trt[:, nl, :, :].rearrange("p v c -> p (v c)")
                        ps = ps1pool.tile([128, 1024], F32, tag="ps1")
                        nc.tensor.matmul(ps[:, 0:512], wat[:, n, :], rhs, start=True, stop=True)
                        nc.tensor.matmul(ps[:, 512:1024], wbt[:, n, :], rhs, start=True, stop=True)
                        if pend_mm2[0] is not None:
                            emit_mm2(*pend_mm2[0])
                        ht = hpool.tile([128, 1024], BF16, tag="h")
                        nc.scalar.activation(ht[:, :], ps[:, :], AFT.Tanh,
                                             bias=biast[:, 0:1], scale=1.0)
                        pend_mm2[0] = (n, ht)
                emit_mm2(*pend_mm2[0])
                pend_sm[0] = scores

            # pipeline: per iteration c emit tail(c-1), out(c-2), stats(c+2), body(c)
            stats_phase(0)
            if n_chunks > 1:
                stats_phase(1)
            for b8 in range(n_chunks):
                if pend_out[0] is not None:
                    emit_out(*pend_out[0])
                    pend_out[0] = None
                if pend_sm[0] is not None:
                    softmax_tail(b8 - 1)
                if b8 + 2 < n_chunks:
                    stats_phase(b8 + 2)
                mm_body(b8)
            # drain: output(n-2), tail(n-1), output(n-1)
            if pend_out[0] is not None:
                emit_out(*pend_out[0])
                pend_out[0] = None
            softmax_tail(n_chunks - 1)
            emit_out(*pend_out[0])
            pend_out[0] = None
    nc.compile()
    return nc


def kernel(**inputs):
    per_core = _host_prep(**inputs)
    if "nc" not in _cached:
        _cached["nc"] = build_nc()
    nc = _cached["nc"]
    trace = bool(os.environ.get("DWA_TRACE"))
    res = run_bass_kernel_spmd(nc, per_core, core_ids=list(range(NCORES)), trace=trace)
    if trace:
        print("HW exec time:", res.exec_time_ns, "ns")
        kernel.last_result = res
    out = np.empty((B_T, N, H), np.float32)
    for c in range(NCORES):
        out[:, c * NC_N:(c + 1) * NC_N, :] = res.results[c]["out"]
    return out
